# revision 11
# baseline (speedup 1.0000x reference)
"""Trainium2 Bass kernel for nn_DAWN_83726092468704 (moe_routing).

Sharding: data-parallel over tokens (B*S=4096 -> 512 per core; core c gets
batch c//2, sequence half c%2). Attention K/V exchanged between the two
cores of each batch via a pairwise AllGather; causal masking is data-driven
(per-core additive mask input) so one SPMD program serves all cores.
Gating (top-8 clusters -> exact top-128 neurons) runs token-major with a
DMA gather of the active cluster blocks and 16 rounds of max8/match_replace
for the exact 128th-largest threshold. Dense sense/emit einsums run in bf16
on the PE with fp32 accumulation. aux is reduced on host from per-core
partials (cluster scores + gathered gate values).
"""

import numpy as np

B, S = 4, 1024
D = 1024
DS = 128
NH, DH = 16, 64
N_QK, N_V, N_KNOW = 4096, 4096, 8192
NC = 64
KC = 8
CS_QK = N_QK // NC            # 64
CS_KNOW = N_KNOW // NC        # 128
NCORES = 8
T = 512
NT = T // 128
P = 128

_CACHE = {}


def _build():
    import concourse.mybir as mybir
    from concourse import bacc
    from concourse.tile import TileContext
    from concourse.library_config import mlp as mlp_lib

    dt = mybir.dt
    F32, BF16 = dt.float32, dt.bfloat16
    AF = mybir.ActivationFunctionType
    ALU = mybir.AluOpType
    AX = mybir.AxisListType

    nc = bacc.Bacc("TRN2", debug=False, num_devices=NCORES)

    def din(name, shape, dtype=F32):
        return nc.dram_tensor(name, shape, dtype, kind="ExternalInput")

    def dout(name, shape, dtype=F32):
        return nc.dram_tensor(name, shape, dtype, kind="ExternalOutput")

    x_in = din("x_sh", [T, D])
    amask_in = din("amask", [T, 2 * T], BF16)
    ln1s = din("ln1s", [1, D]); ln1b = din("ln1b", [1, D])
    ln2s = din("ln2s", [1, D]); ln2b = din("ln2b", [1, D])
    pak = din("pak", [D, 384])
    pab = din("pab", [384, 1])
    tak = din("tak", [D, 3])
    tab = din("tab", [1, 3])
    pkk = din("pkk", [D, DS])
    pkb = din("pkb", [DS, 1])
    tkk = din("tkk", [D, 1])
    tkb = din("tkb", [1, 1])
    embT = {"q": din("embT_qk", [DS, N_QK]), "v": din("embT_v", [DS, N_V]),
            "n": din("embT_kn", [DS, N_KNOW])}
    embT["k"] = embT["q"]
    ceT = {"q": din("ceT_qk", [DS, NC]), "v": din("ceT_v", [DS, NC]),
           "n": din("ceT_kn", [DS, NC])}
    ceT["k"] = ceT["q"]
    # sense weights [n_tile, P(part), 8(k), P(m)] bf16
    nTw = {"q": din("nT_qk", [N_QK // P, P, 8, P], BF16),
           "v": din("nT_v", [N_V // P, P, 8, P], BF16),
           "n": din("nT_kn", [N_KNOW // P, P, 8, P], BF16)}
    nTw["k"] = nTw["q"]
    # emit weights (outT path) [8(d), P(part), n_tile, P(m)] bf16
    Ew = {"q": din("E_qk", [8, P, N_QK // P, P], BF16),
          "n": din("E_kn", [8, P, N_KNOW // P, P], BF16)}
    Ew["k"] = Ew["q"]
    # V emit weights, natural n-tiles [n_tile, P, D] bf16
    Ev2 = din("Ev2", [N_V // P, P, D], BF16)
    O_w = din("O_w", [8, P, D], BF16)
    id_f = din("id_f", [P, P])
    id_b = din("id_b", [P, P], BF16)
    idx0_in = din("idx0", [P, 8], dt.int32)

    y_out = dout("y", [T, D])
    cs_out = {g: dout(f"cs_{g}", [T, NC]) for g in ("q", "k", "v", "n")}
    gg_out = {g: dout(f"gg_{g}", [T, KC * CS_QK]) for g in ("q", "k", "v")}
    gg_out["n"] = dout("gg_n", [T, KC * CS_KNOW])
    cid_out = {g: dout(f"cid_{g}", [T, KC], dt.uint32) for g in ("q", "k", "v", "n")}

    NB = {"q": N_QK // P, "k": N_QK // P, "v": N_V // P, "n": N_KNOW // P}
    CSZ = {"q": CS_QK, "k": CS_QK, "v": CS_QK, "n": CS_KNOW}

    with TileContext(nc) as tc:
        nc.gpsimd.load_library(mlp_lib)
        import contextlib
        with contextlib.ExitStack() as ctx:
            sb = ctx.enter_context(tc.tile_pool(name="sb", bufs=1))
            sb2 = ctx.enter_context(tc.tile_pool(name="sb2", bufs=2))
            sb3 = ctx.enter_context(tc.tile_pool(name="sb3", bufs=3))
            smal = ctx.enter_context(tc.tile_pool(name="smal", bufs=4))
            ps_mm = ctx.enter_context(tc.tile_pool(name="ps_mm", bufs=2, space="PSUM"))
            ps_big = ctx.enter_context(tc.tile_pool(name="ps_big", bufs=2, space="PSUM"))
            ps_t = ctx.enter_context(tc.tile_pool(name="ps_t", bufs=2, space="PSUM"))
            dram = ctx.enter_context(tc.tile_pool(name="dram", bufs=1, space="DRAM"))

            def psmm():
                return ps_mm.tile([P, 512], F32, name="psm", tag="mm")

            def pst_(dtype=F32):
                return ps_t.tile([P, P], dtype, name="pst", tag="pt")

            def psbig():
                return ps_big.tile([P, 1024], F32, name="psb", tag="big")

            # ---------- constants ----------
            idf = sb.tile([P, P], F32)
            nc.sync.dma_start(idf, id_f[:])
            idb = sb.tile([P, P], BF16)
            nc.sync.dma_start(idb, id_b[:])
            idx0_sb = sb.tile([P, 8], dt.int32)
            nc.sync.dma_start(idx0_sb, idx0_in[:])
            one_col = sb.tile([1, P], F32)
            nc.vector.memset(one_col, 1.0)
            ce_sb = {}
            for g in ("q", "v", "n"):
                t_ = sb.tile([P, NC], F32, name=f"ce_{g}")
                nc.sync.dma_start(t_, ceT[g][:])
                ce_sb[g] = t_
            ce_sb["k"] = ce_sb["q"]
            tab_r = sb.tile([P, 3], F32)
            ps0 = pst_()
            rr = smal.tile([1, 4], F32, name="rr", tag="rr")
            nc.sync.dma_start(rr[:, :3], tab[:])
            nc.sync.dma_start(rr[:, 3:4], tkb[:])
            nc.tensor.matmul(ps0[:, :4], one_col, rr, start=True, stop=True)
            tkb_r = sb.tile([P, 1], F32)
            nc.scalar.copy(tab_r, ps0[:, :3])
            nc.scalar.copy(tkb_r, ps0[:, 3:4])
            negone = sb.tile([P, 1], F32)
            nc.vector.memset(negone, -1.0)

            x2D = dram.tile([P, NT, D], F32, name="x2D")

            # ---------- helpers ----------
            def bcast_row(dram_row, name):
                row = sb.tile([1, D], F32, name=f"{name}_rw", tag="brow")
                nc.sync.dma_start(row, dram_row[:])
                out, out_free = tc.tile([P, D], F32, name=name)
                for c0 in range(0, D, 512):
                    ps = psmm()
                    nc.tensor.matmul(ps, one_col, row[:, c0:c0 + 512],
                                     start=True, stop=True)
                    nc.scalar.copy(out[:, c0:c0 + 512], ps)
                return out, out_free

            def ln_into(h, src_fn, s_row, b_row, nm):
                # h: pre-allocated [P, NT, D]; src_fn(t) -> [P, D] f32 AP
                s_r, s_fr = bcast_row(s_row, nm + "s")
                b_r, b_fr = bcast_row(b_row, nm + "b")
                for t in range(NT):
                    xt = src_fn(t)
                    mu = smal.tile([P, 1], F32, name="mu", tag="mu")
                    nc.vector.tensor_reduce(mu, xt, axis=AX.X, op=ALU.add)
                    nc.vector.tensor_scalar_mul(mu, mu, -1.0 / D)
                    xc = sb2.tile([P, D], F32, name="ln_xc", tag="ln_xc", bufs=1)
                    nc.vector.tensor_scalar_add(xc, xt, mu)
                    sq = sb2.tile([P, D], F32, name="ln_sq", tag="ln_sq", bufs=1)
                    var = smal.tile([P, 1], F32, name="var", tag="var")
                    nc.scalar.activation(sq, xc, AF.Square, accum_out=var)
                    nc.vector.tensor_scalar(var, var, 1.0 / D, 1e-6,
                                            op0=ALU.mult, op1=ALU.add)
                    std = smal.tile([P, 1], F32, name="std", tag="std")
                    nc.scalar.activation(std, var, AF.Sqrt)
                    rstd = smal.tile([P, 1], F32, name="rstd", tag="rstd")
                    nc.vector.reciprocal(rstd, std)
                    nc.vector.tensor_scalar_mul(xc, xc, rstd)
                    nc.vector.tensor_mul(xc, xc, s_r)
                    nc.vector.tensor_add(h[:, t, :], xc, b_r)
                b_fr(); s_fr()

            def transpose_into(h, hT, hTb):
                for t in range(NT):
                    for k in range(8):
                        ps = pst_()
                        nc.tensor.transpose(ps, h[:, t, k * P:(k + 1) * P], idf)
                        if hT is not None:
                            nc.scalar.copy(hT[:, k, t * P:(t + 1) * P], ps)
                        nc.vector.tensor_copy(hTb[:, k, t * P:(t + 1) * P], ps)

            def run_gating(g, hT_g, tau_col, tiles=None):
                N, csz, nblk = NB[g] * P, CSZ[g], NB[g]
                A = KC * csz
                tiles = tiles if tiles is not None else list(range(NT))
                gT, gT_fr = tc.tile([P, nblk, P * len(tiles)], BF16, name=f"gT_{g}")
                for ti, t in enumerate(tiles):
                    tsl = slice(t * P, (t + 1) * P)
                    ntau = smal.tile([P, 1], F32, name="ntau", tag="ntau")
                    nc.vector.tensor_scalar_mul(ntau, tau_col[:, t, :], -1.0)
                    # dense scores -> DRAM (chunked through small staging)
                    scD = dram.tile([P, N], F32, name="scD", tag=f"scD_{g}", bufs=2)
                    for c0 in range(0, N, 512):
                        ec = sb3.tile([DS, 512], F32, name="ec", tag="ec")
                        nc.sync.dma_start(ec, embT[g][:, c0:c0 + 512])
                        ps = psmm()
                        nc.tensor.matmul(ps, hT_g[:, tsl], ec, start=True, stop=True)
                        scc = sb3.tile([P, 512], F32, name="scc", tag="scc")
                        nc.scalar.copy(scc, ps)
                        nc.sync.dma_start(scD[:, c0:c0 + 512], scc)
                    # cluster top-8
                    psc = pst_()
                    nc.tensor.matmul(psc[:, :NC], hT_g[:, tsl], ce_sb[g],
                                     start=True, stop=True)
                    cs = smal.tile([P, NC], F32, name="cs_sb", tag="cs_sb")
                    nc.scalar.copy(cs, psc[:, :NC])
                    nc.sync.dma_start(
                        cs_out[g].rearrange("(t p) c -> p t c", p=P)[:, t, :], cs)
                    m8c = smal.tile([P, 8], F32, name="m8c", tag="m8c")
                    nc.vector.max(out=m8c, in_=cs)
                    cmask = smal.tile([P, NC], F32, name="cmask", tag="cmask")
                    nc.vector.tensor_scalar(cmask, cs, m8c[:, 7:8], None,
                                            op0=ALU.is_ge)
                    cid = smal.tile([P, 8], dt.uint32, name="cid", tag="cid")
                    nc.vector.max_index(cid, m8c, cs)
                    nc.sync.dma_start(
                        cid_out[g].rearrange("(t p) c -> p t c", p=P)[:, t, :], cid)
                    # wrapped gather indices
                    idxi = smal.tile([P, 8], dt.int32, name="idxi", tag="idxi")
                    nc.vector.tensor_copy(idxi, cid)
                    nc.vector.tensor_add(idxi, idxi, idx0_sb)
                    idx16 = smal.tile([P, 8], dt.int16, name="idx16", tag="idx16")
                    nc.vector.tensor_copy(idx16, idxi)
                    idxD = dram.tile([KC * P], dt.int16, name="idxD",
                                     tag="idxD", bufs=2)
                    # flat list: idxD[j*128 + p] = idx16[p, j]
                    nc.sync.dma_start(idxD.rearrange("(j p) -> p j", p=P), idx16)
                    idxw = smal.tile([P, 64], dt.int16, name="idxw", tag="idxw")
                    # wrapped [r, c] = idxD[c*16 + r], replicated per 16-row group
                    wsrc = idxD.rearrange("(c s) -> s c", s=16)
                    for a in range(8):
                        nc.sync.dma_start(idxw[16 * a:16 * (a + 1), :], wsrc)
                    # gather active blocks
                    gath = sb2.tile([P, KC, csz], F32, name="gath",
                                    tag=f"gath_{csz}", bufs=1)
                    nc.gpsimd.dma_gather(
                        gath, scD.rearrange("t (c e) -> (t c) e", e=csz),
                        idxw, KC * P, KC * P, csz)
                    # eg = max(exp(score - tau) - 1, 0) on gathered
                    eg = sb2.tile([P, A], F32, name="eg", tag="eg", bufs=1,
                                  padded_shape=[P, KC * CS_KNOW])
                    nc.scalar.activation(eg, gath.rearrange("p a e -> p (a e)"),
                                         AF.Exp, bias=ntau)
                    nc.scalar.activation(eg, eg, AF.Relu, bias=negone)
                    # exact top-128 threshold: 16 rounds of max8
                    egw = sb2.tile([P, A], F32, name="egw", tag="egw", bufs=1,
                                   padded_shape=[P, KC * CS_KNOW])
                    nc.vector.tensor_copy(egw, eg)
                    mx0 = smal.tile([P, 1], F32, name="mx0", tag="mx0")
                    thr = smal.tile([P, 1], F32, name="thr", tag="thr")
                    for r in range(16):
                        m8 = smal.tile([P, 8], F32, name="m8", tag="m8")
                        nc.vector.max(out=m8, in_=egw)
                        if r == 0:
                            nc.vector.tensor_copy(mx0, m8[:, 0:1])
                        if r < 15:
                            nc.vector.match_replace(out=egw, in_to_replace=m8,
                                                    in_values=egw, imm_value=-1.0)
                        else:
                            nc.vector.tensor_copy(thr, m8[:, 7:8])
                    kept = sb2.tile([P, A], F32, name="kept", tag="kept", bufs=1,
                                    padded_shape=[P, KC * CS_KNOW])
                    ssum = smal.tile([P, 1], F32, name="ssum", tag="ssum")
                    nc.vector.scalar_tensor_tensor(kept, eg, thr, eg,
                                                   op0=ALU.is_ge, op1=ALU.mult,
                                                   accum_out=ssum)
                    tnh = smal.tile([P, 1], F32, name="tnh", tag="tnh")
                    nc.scalar.activation(tnh, mx0, AF.Tanh)
                    nc.vector.tensor_scalar_add(ssum, ssum, 1e-8)
                    rec = smal.tile([P, 1], F32, name="rec", tag="rec")
                    nc.vector.reciprocal(rec, ssum)
                    alph = smal.tile([P, 1], F32, name="alph", tag="alph")
                    nc.vector.tensor_mul(alph, tnh, rec)
                    nc.vector.tensor_scalar_mul(kept, kept, alph)
                    nc.sync.dma_start(
                        gg_out[g].rearrange("(t p) a -> p t a", p=P)[:, t, :], kept)
                    # dense gate, streamed in 512-wide chunks
                    thr1 = smal.tile([P, 1], F32, name="thr1", tag="thr1")
                    nc.vector.tensor_scalar_add(thr1, thr, 1.0)
                    cma = smal.tile([P, NC], F32, name="cma", tag="cma")
                    nc.vector.tensor_scalar_mul(cma, cmask, alph)
                    cpb = NC // (N // 512)
                    for ci, c0 in enumerate(range(0, N, 512)):
                        scl = sb3.tile([P, 512], F32, name="scl", tag="scc")
                        nc.sync.dma_start(scl, scD[:, c0:c0 + 512])
                        edc = sb3.tile([P, 512], F32, name="edc", tag="edc")
                        nc.scalar.activation(edc, scl, AF.Exp, bias=ntau)
                        nc.vector.scalar_tensor_tensor(edc, edc, thr1, edc,
                                                       op0=ALU.is_ge, op1=ALU.mult)
                        edc2 = sb3.tile([P, 512], F32, name="edc2", tag="edc")
                        nc.scalar.activation(edc2, edc, AF.Relu, bias=negone)
                        edc = edc2
                        gdc = sb3.tile([P, 512], BF16, name="gdc", tag="gdc")
                        nc.vector.tensor_tensor(
                            gdc.rearrange("p (c e) -> p c e", e=csz),
                            edc.rearrange("p (c e) -> p c e", e=csz),
                            cma[:, ci * cpb:(ci + 1) * cpb].rearrange(
                                "p (c o) -> p c o", o=1).to_broadcast(
                                    [P, cpb, csz]),
                            op=ALU.mult)
                        ps4 = ps_mm.tile([P, 512], BF16, name="ps4", tag="mm")
                        for q in range(4):
                            nc.tensor.transpose(
                                ps4[:, q * P:(q + 1) * P],
                                gdc[:, q * P:(q + 1) * P], idb)
                        nc.vector.tensor_copy(
                            gT[:, ci * 4:(ci + 1) * 4,
                               ti * P:(ti + 1) * P],
                            ps4.rearrange("p (q t) -> p q t", q=4))
                return gT, gT_fr

            def sense(g, gT, hTb_src, ts):
                nblk = NB[g]
                Tw = gT.shape[2]
                for nb in range(nblk):
                    w = sb3.tile([P, 8, P], BF16, name="w_nT", tag="w_nT")
                    nc.sync.dma_start(w, nTw[g][nb])
                    ps = psmm()
                    for k in range(8):
                        nc.tensor.matmul(ps[:, :Tw], w[:, k, :], hTb_src[:, k, ts],
                                         start=(k == 0), stop=(k == 7))
                    nc.vector.tensor_tensor(gT[:, nb, :], ps[:, :Tw], gT[:, nb, :],
                                            op=ALU.mult)

            def emit_T(g, gT, store_fn, ts):
                # outT path: store_fn(d, ts, sbuf_tile) consumes [P, Tw] rows d
                nblk = NB[g]
                Tw = gT.shape[2]
                nh = nblk // 4
                for d in range(8):
                    pse = psmm()
                    for hf in range(4):
                        we = sb2.tile([P, nh, P], BF16, name="w_E", tag="w_E",
                                      padded_shape=[P, N_KNOW // P // 4, P])
                        nc.sync.dma_start(we, Ew[g][d][:, hf * nh:(hf + 1) * nh, :])
                        for nb in range(nh):
                            gnb = hf * nh + nb
                            nc.tensor.matmul(pse[:, :Tw], we[:, nb, :], gT[:, gnb, :],
                                             start=(gnb == 0), stop=(gnb == nblk - 1))
                    store_fn(d, ts, pse[:, :Tw])

            # =======================================================
            # main flow  (singles allocated in reverse-death order)
            # =======================================================
            QoT, QoT_fr = tc.tile([P, 8, T], BF16, name="QoT")
            h1Tb, h1Tb_fr = tc.tile([P, 8, T], BF16, name="h1Tb")
            h1T, h1T_fr = tc.tile([P, 8, T], F32, name="h1T")
            h1, h1_fr = tc.tile([P, NT, D], F32, name="h1")
            x_sb, x_fr = tc.tile([P, NT, D], F32, name="x_sb")
            nc.sync.dma_start(x_sb, x_in.rearrange("(t p) d -> p t d", p=P))
            ln_into(h1, lambda t: x_sb[:, t, :], ln1s, ln1b, "l1")
            x_fr()
            transpose_into(h1, h1T, h1Tb)
            h1_fr()

            tau_sb, tau_fr = tc.tile([P, NT, 3], F32, name="tau_sb")
            hvT, hvT_fr = tc.tile([P, T], F32, name="hvT")
            hkT, hkT_fr = tc.tile([P, T], F32, name="hkT")
            hqT, hqT_fr = tc.tile([P, T], F32, name="hqT")
            pak_sb, pak_fr = tc.tile([P, 8, 384], F32, name="pak_sb")
            nc.sync.dma_start(pak_sb, pak.rearrange("(a p) m -> p a m", p=P))
            pab_sb, pab_fr = tc.tile([P, 3], F32, name="pab_sb")
            nc.sync.dma_start(pab_sb, pab.rearrange("(m p) o -> p (m o)", p=P))
            tak_sb, tak_fr = tc.tile([P, 8, 3], F32, name="tak_sb")
            nc.sync.dma_start(tak_sb, tak.rearrange("(a p) m -> p a m", p=P))
            hgT = {"q": hqT, "k": hkT, "v": hvT}
            for m, g in enumerate(("q", "k", "v")):
                ps = psmm()
                for k in range(8):
                    nc.tensor.matmul(ps, pak_sb[:, k, m * P:(m + 1) * P],
                                     h1T[:, k, :], start=(k == 0), stop=(k == 7))
                nc.vector.tensor_scalar_add(hgT[g], ps, pab_sb[:, m:m + 1])
            for t in range(NT):
                ps = pst_()
                for k in range(8):
                    nc.tensor.matmul(ps[:, :3], h1T[:, k, t * P:(t + 1) * P],
                                     tak_sb[:, k, :], start=(k == 0), stop=(k == 7))
                nc.vector.tensor_add(tau_sb[:, t, :], ps[:, :3], tab_r)
            tak_fr(); pab_fr(); pak_fr()

            cc_in = dram.tile([P, 8192], BF16, name="cc_in")

            # --- Q ---
            gT_q, gT_q_fr = run_gating("q", hqT, tau_sb[:, :, 0:1])
            sense("q", gT_q, h1Tb, slice(0, T))

            def store_q(d, ts, pse):
                nc.scalar.copy(QoT[:, d, ts], pse)
            emit_T("q", gT_q, store_q, slice(0, T))
            gT_q_fr(); hqT_fr()

            # --- K (emit straight to cc_in) ---
            gT_k, gT_k_fr = run_gating("k", hkT, tau_sb[:, :, 1:2])
            sense("k", gT_k, h1Tb, slice(0, T))

            def store_k(d, ts, pse):
                ko = sb3.tile([P, T], BF16, name="ko", tag="ko")
                nc.vector.tensor_copy(ko, pse)
                nc.sync.dma_start(cc_in[:, d * T:(d + 1) * T], ko)
            emit_T("k", gT_k, store_k, slice(0, T))
            gT_k_fr(); hkT_fr()

            # --- V (token-major emit straight to cc_in) ---
            gT_v, gT_v_fr = run_gating("v", hvT, tau_sb[:, :, 2:3])
            sense("v", gT_v, h1Tb, slice(0, T))
            for half in range(2):
                psV = [psbig() for _ in range(2)]
                for nb in range(N_V // P):
                    wv = sb2.tile([P, D], BF16, name="wv", tag="w_E")
                    nc.sync.dma_start(wv, Ev2[nb])
                    for i in range(2):
                        t = half * 2 + i
                        for c in range(2):
                            nc.tensor.matmul(psV[i][:, c * 512:(c + 1) * 512],
                                             gT_v[:, nb, t * P:(t + 1) * P],
                                             wv[:, c * 512:(c + 1) * 512],
                                             start=(nb == 0),
                                             stop=(nb == N_V // P - 1))
                for i in range(2):
                    t = half * 2 + i
                    vo = sb3.tile([P, D], BF16, name="vo", tag="vo")
                    nc.vector.tensor_copy(vo, psV[i])
                    nc.sync.dma_start(cc_in[:, 4096 + t * D:4096 + (t + 1) * D], vo)
            gT_v_fr(); hvT_fr(); tau_fr(); h1T_fr(); h1Tb_fr()

            # ---------- K/V exchange ----------
            cc_out = dram.tile([2, P, 8192], BF16, name="cc_out")
            nc.gpsimd.collective_compute(
                "AllGather", ALU.bypass,
                replica_groups=[[0, 1], [2, 3], [4, 5], [6, 7]],
                ins=[cc_in.opt()], outs=[cc_out.opt()])

            # ---------- attention ----------
            attT, attT_fr = tc.tile([P, 8, T], BF16, name="attT")
            Kw, Kw_fr = tc.tile([P, 2, 8, T], BF16, name="Kw")
            Vw, Vw_fr = tc.tile([P, 2, NT, D], BF16, name="Vw")
            for w in range(2):
                nc.sync.dma_start(Kw[:, w].rearrange("p a t -> p (a t)"),
                                  cc_out[w, :, :4096])
                nc.sync.dma_start(Vw[:, w].rearrange("p t d -> p (t d)"),
                                  cc_out[w, :, 4096:])
            amask_sb, am_fr = tc.tile([P, NT, 2 * T], BF16, name="amask_sb")
            nc.sync.dma_start(amask_sb, amask_in.rearrange("(t p) k -> p t k", p=P))
            for h in range(NH):
                dtl, prow = h // 2, (h % 2) * 64
                for t in range(NT):
                    tsl = slice(t * P, (t + 1) * P)
                    psS = psbig()
                    for w in range(2):
                        nc.tensor.matmul(psS[:, w * T:(w + 1) * T],
                                         QoT[prow:prow + 64, dtl, tsl],
                                         Kw[prow:prow + 64, w, dtl, :],
                                         start=True, stop=True)
                    nc.vector.tensor_add(psS, psS, amask_sb[:, t, :])
                    mxs = smal.tile([P, 1], F32, name="mxs", tag="mxs")
                    nc.vector.tensor_reduce(mxs, psS, axis=AX.X, op=ALU.max)
                    nc.vector.tensor_scalar_mul(mxs, mxs, -0.125)
                    p_sb = sb2.tile([P, 2 * T], BF16, name="p_sb", tag="p_sb")
                    sm = smal.tile([P, 1], F32, name="sm", tag="sm")
                    nc.scalar.activation(p_sb, psS, AF.Exp, bias=mxs,
                                         scale=0.125, accum_out=sm)
                    rs = smal.tile([P, 1], F32, name="rs", tag="rs")
                    nc.vector.reciprocal(rs, sm)
                    nc.vector.tensor_scalar_mul(p_sb, p_sb, rs)
                    psAV = ps_mm.tile([64, P], F32, name="psAV", tag="mm")
                    for qq in range(2):
                        ps4 = ps_mm.tile([P, 512], BF16, name="ps4", tag="mm")
                        for q in range(4):
                            j = qq * 4 + q
                            nc.tensor.transpose(ps4[:, q * P:(q + 1) * P],
                                                p_sb[:, j * P:(j + 1) * P], idb)
                        at_bf = sb3.tile([P, 512], BF16, name="at_bf", tag="at_bf")
                        nc.vector.tensor_copy(at_bf, ps4)
                        for q in range(4):
                            j = qq * 4 + q
                            nc.tensor.matmul(psAV,
                                             Vw[:, j // 4, j % 4,
                                                h * 64:(h + 1) * 64],
                                             at_bf[:, q * P:(q + 1) * P],
                                             start=(j == 0), stop=(j == 7))
                    nc.scalar.copy(attT[prow:prow + 64, dtl, tsl], psAV)
            am_fr(); Vw_fr(); Kw_fr()

            # ---------- expand_O + residual -> x2D ----------
            O_sb, O_fr = tc.tile([P, 8, D], BF16, name="O_sb")
            nc.sync.dma_start(O_sb, O_w.rearrange("a p d -> p a d"))
            for t in range(NT):
                psO = psbig()
                for k in range(8):
                    for c in range(2):
                        nc.tensor.matmul(psO[:, c * 512:(c + 1) * 512],
                                         attT[:, k, t * P:(t + 1) * P],
                                         O_sb[:, k, c * 512:(c + 1) * 512],
                                         start=(k == 0), stop=(k == 7))
                xr = sb2.tile([P, D], F32, name="xr", tag="xr", bufs=1)
                nc.sync.dma_start(xr, x_in.rearrange("(t p) d -> p t d", p=P)[:, t, :])
                x2t = sb2.tile([P, D], F32, name="x2t", tag="x2t", bufs=1)
                nc.vector.tensor_add(x2t, psO, xr)
                nc.sync.dma_start(x2D[:, t, :], x2t)
            O_fr(); attT_fr(); QoT_fr()

            # ---------- knowledge circuit ----------
            NoT, NoT_fr = tc.tile([P, 8, T], BF16, name="NoT")
            h2Tb, h2Tb_fr = tc.tile([P, 8, T], BF16, name="h2Tb")
            h2T, h2T_fr = tc.tile([P, 8, T], F32, name="h2T")
            h2, h2_fr = tc.tile([P, NT, D], F32, name="h2")

            def x2_src(t):
                x2t = sb2.tile([P, D], F32, name="x2l", tag="x2t", bufs=1)
                nc.sync.dma_start(x2t, x2D[:, t, :])
                return x2t
            ln_into(h2, x2_src, ln2s, ln2b, "l2")
            transpose_into(h2, h2T, h2Tb)
            h2_fr()

            hNT, hNT_fr = tc.tile([P, T], F32, name="hNT")
            tau_n, tau_n_fr = tc.tile([P, NT, 1], F32, name="tau_n")
            pkk_sb, pkk_fr = tc.tile([P, 8, DS], F32, name="pkk_sb")
            nc.sync.dma_start(pkk_sb, pkk.rearrange("(a p) m -> p a m", p=P))
            pkb_sb, pkb_fr = tc.tile([P, 1], F32, name="pkb_sb")
            nc.sync.dma_start(pkb_sb, pkb[:])
            tkk_sb, tkk_fr = tc.tile([P, 8, 1], F32, name="tkk_sb")
            nc.sync.dma_start(tkk_sb, tkk.rearrange("(a p) m -> p a m", p=P))
            ps = psmm()
            for k in range(8):
                nc.tensor.matmul(ps, pkk_sb[:, k, :], h2T[:, k, :],
                                 start=(k == 0), stop=(k == 7))
            nc.vector.tensor_scalar_add(hNT, ps, pkb_sb)
            for t in range(NT):
                pstt = pst_()
                for k in range(8):
                    nc.tensor.matmul(pstt[:, :1], h2T[:, k, t * P:(t + 1) * P],
                                     tkk_sb[:, k, :], start=(k == 0), stop=(k == 7))
                nc.vector.tensor_add(tau_n[:, t, :], pstt[:, :1], tkb_r)
            tkk_fr(); pkb_fr(); pkk_fr()

            for hf in range(2):
                tls = [2 * hf, 2 * hf + 1]
                ts = slice(hf * 2 * P, (hf + 1) * 2 * P)
                gT_n, gT_n_fr = run_gating("n", hNT, tau_n, tiles=tls)
                sense("n", gT_n, h2Tb, ts)

                def store_n(d, tss, pse):
                    nc.scalar.copy(NoT[:, d, tss], pse)
                emit_T("n", gT_n, store_n, ts)
                gT_n_fr()
            tau_n_fr(); hNT_fr(); h2T_fr(); h2Tb_fr()

            # know out -> token-major + residual -> y
            for t in range(NT):
                yt = sb2.tile([P, D], F32, name="yt", tag="yt", bufs=1)
                xr2 = sb2.tile([P, D], F32, name="xr2", tag="x2t", bufs=1)
                nc.sync.dma_start(xr2, x2D[:, t, :])
                for dtl in range(8):
                    ps = pst_(BF16)
                    nc.tensor.transpose(ps, NoT[:, dtl, t * P:(t + 1) * P], idb)
                    nc.vector.tensor_add(yt[:, dtl * P:(dtl + 1) * P], ps,
                                         xr2[:, dtl * P:(dtl + 1) * P])
                nc.sync.dma_start(
                    y_out.rearrange("(t p) d -> p t d", p=P)[:, t, :], yt)
            NoT_fr()

    nc.compile()
    return nc


def _prep_inputs(inputs):
    import ml_dtypes
    bf16 = ml_dtypes.bfloat16
    f32 = np.float32

    emb = np.asarray(inputs["neuron_emb"], f32)
    emb_n = emb / (np.linalg.norm(emb, axis=-1, keepdims=True) + 1e-8)
    qk_emb, v_emb, kn_emb = emb_n[:N_QK], emb_n[N_QK:N_QK + N_V], emb_n[N_QK + N_V:]

    def ce_norm(c):
        c = np.asarray(c, f32)
        return c / (np.linalg.norm(c, axis=-1, keepdims=True) + 1e-8)

    def sense_layout(neur):   # [N, D] -> [N/P, P(part), 8(k), P(m)]
        n = neur.shape[0]
        return np.ascontiguousarray(
            neur.reshape(n // P, P, 8, P).transpose(0, 3, 2, 1)).astype(bf16)

    def emit_layout(neur):    # [N, D] -> [8(d), P(part), N/P, P(m)]
        n = neur.shape[0]
        return np.ascontiguousarray(
            neur.reshape(n // P, P, 8, P).transpose(2, 1, 0, 3)).astype(bf16)

    qk_n = np.asarray(inputs["qk_neurons"], f32)
    v_n = np.asarray(inputs["v_neurons"], f32)
    kn_n = np.asarray(inputs["know_neurons"], f32)

    common = {
        "ln1s": np.asarray(inputs["ln1_scale"], f32).reshape(1, D),
        "ln1b": np.asarray(inputs["ln1_bias"], f32).reshape(1, D),
        "ln2s": np.asarray(inputs["ln2_scale"], f32).reshape(1, D),
        "ln2b": np.asarray(inputs["ln2_bias"], f32).reshape(1, D),
        "pak": np.asarray(inputs["proj_attn_k"], f32),
        "pab": np.asarray(inputs["proj_attn_b"], f32).reshape(384, 1),
        "tak": np.asarray(inputs["tau_attn_k"], f32),
        "tab": np.asarray(inputs["tau_attn_b"], f32).reshape(1, 3),
        "pkk": np.asarray(inputs["proj_know_k"], f32),
        "pkb": np.asarray(inputs["proj_know_b"], f32).reshape(DS, 1),
        "tkk": np.asarray(inputs["tau_know_k"], f32),
        "tkb": np.asarray(inputs["tau_know_b"], f32).reshape(1, 1),
        "embT_qk": np.ascontiguousarray(qk_emb.T),
        "embT_v": np.ascontiguousarray(v_emb.T),
        "embT_kn": np.ascontiguousarray(kn_emb.T),
        "ceT_qk": np.ascontiguousarray(ce_norm(inputs["cluster_emb_qk"]).T),
        "ceT_v": np.ascontiguousarray(ce_norm(inputs["cluster_emb_v"]).T),
        "ceT_kn": np.ascontiguousarray(ce_norm(inputs["cluster_emb_know"]).T),
        "nT_qk": sense_layout(qk_n), "nT_v": sense_layout(v_n),
        "nT_kn": sense_layout(kn_n),
        "E_qk": emit_layout(qk_n), "E_kn": emit_layout(kn_n),
        "Ev2": np.ascontiguousarray(v_n.reshape(N_V // P, P, D)).astype(bf16),
        "O_w": np.ascontiguousarray(
            np.asarray(inputs["expand_O"], f32).reshape(8, P, D)).astype(bf16),
        "id_f": np.eye(P, dtype=f32),
        "id_b": np.eye(P, dtype=f32).astype(bf16),
        "idx0": np.repeat(np.arange(P, dtype=np.int32)[:, None] * 64, 8, axis=1),
    }

    x = np.asarray(inputs["x"], f32).reshape(B * S, D)
    in_maps = []
    for c in range(NCORES):
        half = c % 2
        qg = half * T + np.arange(T)
        kv = np.arange(2 * T)
        am = np.where(kv[None, :] <= qg[:, None], 0.0, -1e30).astype(bf16)
        m = dict(common)
        m["x_sh"] = np.ascontiguousarray(x[c * T:(c + 1) * T])
        m["amask"] = am
        in_maps.append(m)
    return in_maps


def _aux_from_outputs(results):
    f32 = np.float32
    aux = 0.0
    for g, N, csz in (("q", N_QK, CS_QK), ("k", N_QK, CS_QK),
                      ("v", N_V, CS_QK), ("n", N_KNOW, CS_KNOW)):
        cs = np.concatenate([r[f"cs_{g}"] for r in results], 0).astype(f32)
        csm = cs - cs.max(-1, keepdims=True)
        p = np.exp(csm)
        p /= p.sum(-1, keepdims=True)
        freq = p.mean(0)
        aux += ((freq - 1.0 / NC) ** 2).sum() * NC
        gg = np.concatenate([r[f"gg_{g}"] for r in results], 0).astype(np.float64)
        cid = np.concatenate([r[f"cid_{g}"] for r in results], 0).astype(np.int64)
        ids = (cid[:, :, None] * csz + np.arange(csz)[None, None, :]).reshape(
            gg.shape[0], -1)
        nfreq = np.bincount(ids.ravel(), weights=gg.ravel(),
                            minlength=N)[:N] / gg.shape[0]
        aux += ((nfreq - 1.0 / N) ** 2).sum() * N
    return np.float32(aux)


def kernel(**inputs):
    if "nc" not in _CACHE:
        _CACHE["nc"] = _build()
    nc = _CACHE["nc"]
    from concourse import bass_utils
    in_maps = _prep_inputs(inputs)
    res = bass_utils.run_bass_kernel_spmd(nc, in_maps, core_ids=list(range(NCORES)))
    results = res.results
    y = np.concatenate([r["y"] for r in results], 0).reshape(B, S, D).astype(np.float32)
    aux = _aux_from_outputs(results)
    return y, aux
